# revision 1
# baseline (speedup 1.0000x reference)
"""Trainium2 Bass kernel for MixtureOfSoftmaxes.

Module: RMSNorm -> gate MLP (silu, softmax over K experts) -> big GEMM
x @ expert_w (H=1024 -> K*V=128000), softmax over V per expert, mix with
gate weights, log.

Sharding: tensor-parallel over vocab. Core c computes, for ALL K=4
experts, the vocab window [c*4000, (c+1)*4000) (padded to 4096 per
expert). The only cross-core quantity is the per-(token, expert) softmax
denominator Z = sum_v exp(logit); each core emits its local partial sums
and the host reduces them between the two launches (16 KB total).
Logits are bounded (|l| < ~5 for this distribution), so exp() without
max-subtraction is numerically safe.

Phase A (per core): RMSNorm, gate MLP -> gate logits; big GEMM in bf16
  -> P = exp(logits) (bf16, spilled to HBM) + row-sum partials S.
Host: Z = sum_c S_c - pad_correction; a[t,k] = softmax(gate)[t,k] / Z[t,k].
Phase B (per core): out[t, v] = log(sum_k a[t,k] * P[t,k,v] + 1e-10) for
  its vocab window; host concatenates windows.
"""

import os
import sys

sys.path.insert(0, "/opt/trn_rl_repo")

import numpy as np
import ml_dtypes

import concourse.bass as bass
import concourse.bacc as bacc
import concourse.mybir as mybir
import concourse.tile as tile
from concourse.bass_utils import run_bass_kernel_spmd
from concourse.masks import make_identity

AFT = mybir.ActivationFunctionType
F32 = mybir.dt.float32
BF16 = mybir.dt.bfloat16
FP8 = mybir.dt.float8e4
FP8NP = ml_dtypes.float8_e4m3
WSCALE = 16.0

B, S, H, K, V = 2, 512, 1024, 4, 32000
T = B * S              # 1024 tokens
NC = 8                 # cores
VSH = V // NC          # 4000 vocab cols per core per expert
VP = 4096              # padded per-expert width
C = K * VP             # 16384 GEMM cols per core
D = H // 2             # 512 gate hidden
EPS_RMS = 1e-5
EPS_LOG = 1e-10
TB = T // 128          # 8 token blocks
HB = H // 128          # 8 contraction blocks
NG = C // 512          # 32 col groups of 512
GPE = VP // 512        # 8 groups per expert


def build_phase_a():
    nc = bacc.Bacc("TRN2", target_bir_lowering=False, debug=False, num_devices=NC)
    x_d = nc.dram_tensor("x", [T, H], F32, kind="ExternalInput")
    w_d = nc.dram_tensor("w", [H, C], FP8, kind="ExternalInput")
    wd_d = nc.dram_tensor("wd", [H, D], BF16, kind="ExternalInput")
    wu_d = nc.dram_tensor("wu", [D, K], BF16, kind="ExternalInput")
    bd_d = nc.dram_tensor("bd", [D, 1], F32, kind="ExternalInput")
    bu_d = nc.dram_tensor("bu", [K, 1], F32, kind="ExternalInput")
    p_d = nc.dram_tensor("p", [TB, 128, C], BF16, kind="ExternalOutput")
    s_d = nc.dram_tensor("s", [TB, 128, K], F32, kind="ExternalOutput")
    gl_d = nc.dram_tensor("gl", [K, T], F32, kind="ExternalOutput")

    x_ap = x_d.rearrange("(t p) h -> t p h", p=128)
    w_ap8 = w_d.rearrange("(hs j p) c -> hs p j c", j=2, p=128)
    wd_ap = wd_d.rearrange("(hb p) d -> p hb d", p=128)
    wu_ap = wu_d.rearrange("(db p) k -> p db k", p=128)
    bd_ap = bd_d.rearrange("(db p) o -> p db o", p=128)

    with tile.TileContext(nc) as tc:
        with tc.tile_pool(name="persist", bufs=1) as pers, \
             tc.tile_pool(name="norm", bufs=2) as norm_pool:
            # ---- load + RMSNorm + transpose to xT (h on partitions) ----
            ident = pers.tile([128, 128], BF16)
            make_identity(nc, ident[:])
            eps_rms = pers.tile([128, 1], F32)
            nc.gpsimd.memset(eps_rms[:], EPS_RMS)
            xT = pers.tile([128, HB, T], BF16)       # 16 KB/partition (gate)
            xT8 = pers.tile([128, HB, T], FP8)       # 8 KB/partition (big GEMM)
            ss = pers.tile([128, TB], F32)
            sd = pers.tile([128, TB], F32)
            rinv = pers.tile([128, TB], F32)
            dummy = pers.tile([128, 1], F32)

            with tc.tile_pool(name="tp_psum", bufs=2, space="PSUM") as tp_psum:
                for t in range(TB):
                    xt = norm_pool.tile([128, H], F32, tag="xt")
                    nc.sync.dma_start(xt[:], x_ap[t])
                    # sum of squares via ACT Square + accum_out
                    sq = norm_pool.tile([128, H], F32, tag="sq")
                    nc.scalar.activation(sq[:], xt[:], AFT.Square, bias=0.0,
                                         scale=1.0, accum_out=ss[:, t : t + 1])
                    nc.scalar.activation(sd[:, t : t + 1], ss[:, t : t + 1],
                                         AFT.Sqrt, bias=eps_rms[:], scale=1.0 / H)
                    nc.vector.reciprocal(rinv[:, t : t + 1], sd[:, t : t + 1])
                    xb = norm_pool.tile([128, H], BF16, tag="xb")
                    nc.scalar.mul(xb[:], xt[:], rinv[:, t : t + 1])
                    if not os.environ.get("KSKIP_TRANS"):
                      for h in range(HB):
                        tp = tp_psum.tile([128, 128], BF16, tag="tp")
                        nc.tensor.transpose(tp[:], xb[:, h * 128 : (h + 1) * 128], ident[:])
                        nc.vector.tensor_copy(xT[:, h, t * 128 : (t + 1) * 128], tp[:])
                        nc.scalar.copy(xT8[:, h, t * 128 : (t + 1) * 128], tp[:])

            # ---- gate MLP ----
            if os.environ.get("KSKIP_GATE"):
                gq = pers.tile([K, T], F32)
                nc.gpsimd.memset(gq[:], 0.0)
                nc.sync.dma_start(gl_d[:], gq[:])
            elif True:
              with tc.tile_pool(name="gate_sb", bufs=1) as gsb, \
                   tc.tile_pool(name="gate_psum", bufs=2, space="PSUM") as gps:
                  wd_sb = gsb.tile([128, HB, D], BF16)   # 8 KB/partition
                  nc.sync.dma_start(wd_sb[:], wd_ap)
                  wu_sb = gsb.tile([128, D // 128, K], BF16)
                  nc.sync.dma_start(wu_sb[:], wu_ap)
                  bd_sb = gsb.tile([128, D // 128, 1], F32)
                  nc.sync.dma_start(bd_sb[:], bd_ap)
                  bu_sb = gsb.tile([K, 1], F32)
                  nc.sync.dma_start(bu_sb[:], bu_d[:])
                  gT = gsb.tile([128, D // 128, T], BF16)
                  for d in range(D // 128):
                      pg = gps.tile([128, T], F32, tag="pg")  # 2 banks
                      for h in range(HB):
                          for half in range(2):
                              nc.tensor.matmul(
                                  pg[:, half * 512 : (half + 1) * 512],
                                  lhsT=wd_sb[:, h, d * 128 : (d + 1) * 128],
                                  rhs=xT[:, h, half * 512 : (half + 1) * 512],
                                  start=(h == 0), stop=(h == HB - 1),
                              )
                      lin = gsb.tile([128, T], F32, tag="lin", name=f"lin{d}")
                      nc.scalar.activation(lin[:], pg[:], AFT.Identity,
                                           bias=bd_sb[:, d, :], scale=1.0)
                      sig = gsb.tile([128, T], F32, tag="sig", name=f"sig{d}")
                      nc.scalar.activation(sig[:], pg[:], AFT.Sigmoid,
                                           bias=bd_sb[:, d, :], scale=1.0)
                      nc.vector.tensor_mul(gT[:, d, :], lin[:], sig[:])
                  pl = gps.tile([K, T], F32, tag="pl")
                  for d in range(D // 128):
                      for half in range(2):
                          nc.tensor.matmul(
                              pl[:, half * 512 : (half + 1) * 512],
                              lhsT=wu_sb[:, d, :],
                              rhs=gT[:, d, half * 512 : (half + 1) * 512],
                              start=(d == 0), stop=(d == D // 128 - 1),
                          )
                  gl_sb = gsb.tile([K, T], F32)
                  nc.scalar.activation(gl_sb[:], pl[:], AFT.Identity,
                                       bias=bu_sb[:], scale=1.0)
                  nc.sync.dma_start(gl_d[:], gl_sb[:])

            # ---- big GEMM + exp + partial sums ----
            schunk = pers.tile([128, TB, NG], F32)
            if os.environ.get("KSKIP_GEMM"):
                nc.gpsimd.memset(schunk[:], 1.0)
            elif True:
              with tc.tile_pool(name="wmm", bufs=3) as wpool, \
                   tc.tile_pool(name="pout", bufs=4) as ppool, \
                   tc.tile_pool(name="mm_psum", bufs=8, space="PSUM") as mmps:
                  for ha in range(2):          # token halves; W streamed twice
                      for g in range(NG):
                          psums = []
                          for t4 in range(4):
                              psums.append(mmps.tile([128, 512], F32, tag="mm",
                                                     name=f"mm_{ha}_{g}_{t4}"))
                          for hs in range(HB // 2):
                              wt = wpool.tile([128, 2, 512], FP8, tag="wt")
                              nc.sync.dma_start(wt[:], w_ap8[hs, :, :, g * 512 : (g + 1) * 512])
                              for t4 in range(4):
                                  t = ha * 4 + t4
                                  nc.tensor.matmul(
                                      psums[t4][:],
                                      lhsT=xT8[:, 2 * hs : 2 * hs + 2, t * 128 : (t + 1) * 128],
                                      rhs=wt[:],
                                      start=(hs == 0), stop=(hs == HB // 2 - 1),
                                      perf_mode=mybir.MatmulPerfMode.DoubleRow,
                                  )
                          for t4 in range(4):
                              t = ha * 4 + t4
                              pt = ppool.tile([128, 512], BF16, tag="pt")
                              nc.scalar.activation(pt[:], psums[t4][:], AFT.Exp,
                                                   bias=0.0, scale=1.0 / WSCALE,
                                                   accum_out=schunk[:, t, g : g + 1])
                              nc.sync.dma_start(p_d[t, :, g * 512 : (g + 1) * 512], pt[:])

            # ---- reduce partials to per-expert sums ----
            s_sb = pers.tile([128, TB, K], F32)
            nc.vector.tensor_reduce(
                s_sb[:], schunk[:].rearrange("p t (k g) -> p t k g", g=GPE),
                axis=mybir.AxisListType.X, op=mybir.AluOpType.add,
            )
            if not os.environ.get("KSKIP_SOUT"):
                nc.sync.dma_start(s_d.rearrange("t p k -> p t k"), s_sb[:])
            else:
                nc.sync.dma_start(s_d[0], s_sb[:, 0, :])
    nc.compile()
    return nc



def build_fused():
    nc = bacc.Bacc("TRN2", target_bir_lowering=False, debug=False, num_devices=NC)
    x_d = nc.dram_tensor("x", [T, H], F32, kind="ExternalInput")
    w_d = nc.dram_tensor("w", [H, C], FP8, kind="ExternalInput")
    wd_d = nc.dram_tensor("wd", [H, D], BF16, kind="ExternalInput")
    wu_d = nc.dram_tensor("wu", [D, K], BF16, kind="ExternalInput")
    bd_d = nc.dram_tensor("bd", [D, 1], F32, kind="ExternalInput")
    bu_d = nc.dram_tensor("bu", [K, 1], F32, kind="ExternalInput")
    o_d = nc.dram_tensor("o", [TB, 128, VSH], F32, kind="ExternalOutput")

    x_ap = x_d.rearrange("(t p) h -> t p h", p=128)
    w_ap8 = w_d.rearrange("(hs j p) c -> hs p j c", j=2, p=128)
    wd_ap = wd_d.rearrange("(hb p) d -> p hb d", p=128)
    wu_ap = wu_d.rearrange("(db p) k -> p db k", p=128)
    bd_ap = bd_d.rearrange("(db p) o -> p db o", p=128)

    with tile.TileContext(nc) as tc:
        with tc.tile_pool(name="persist", bufs=1) as pers:
            ident = pers.tile([128, 128], BF16)
            make_identity(nc, ident[:])
            ident32 = pers.tile([128, 128], F32)
            make_identity(nc, ident32[:])
            eps_rms = pers.tile([128, 1], F32)
            nc.gpsimd.memset(eps_rms[:], EPS_RMS)
            eps_log = pers.tile([128, 1], F32)
            nc.gpsimd.memset(eps_log[:], EPS_LOG)
            xT = pers.tile([128, HB, T], BF16)
            xT8 = pers.tile([128, HB, T], FP8)
            ss = pers.tile([128, TB], F32)
            sd = pers.tile([128, TB], F32)
            rinv = pers.tile([128, TB], F32)
            gw = pers.tile([128, TB, K], F32)

            # ---- RMSNorm + transpose ----
            with tc.tile_pool(name="norm", bufs=2) as norm_pool, \
                 tc.tile_pool(name="tp_psum", bufs=2, space="PSUM") as tp_psum:
                for t in range(TB):
                    xt = norm_pool.tile([128, H], F32, tag="xt")
                    nc.sync.dma_start(xt[:], x_ap[t])
                    sq = norm_pool.tile([128, H], F32, tag="sq")
                    nc.scalar.activation(sq[:], xt[:], AFT.Square, bias=0.0,
                                         scale=1.0, accum_out=ss[:, t : t + 1])
                    nc.scalar.activation(sd[:, t : t + 1], ss[:, t : t + 1],
                                         AFT.Sqrt, bias=eps_rms[:], scale=1.0 / H)
                    nc.vector.reciprocal(rinv[:, t : t + 1], sd[:, t : t + 1])
                    xb = norm_pool.tile([128, H], BF16, tag="xb")
                    nc.scalar.mul(xb[:], xt[:], rinv[:, t : t + 1])
                    for h in range(HB):
                        tp = tp_psum.tile([128, 128], BF16, tag="tp")
                        nc.tensor.transpose(tp[:], xb[:, h * 128 : (h + 1) * 128], ident[:])
                        nc.vector.tensor_copy(xT[:, h, t * 128 : (t + 1) * 128], tp[:])
                        nc.scalar.copy(xT8[:, h, t * 128 : (t + 1) * 128], tp[:])

            # ---- gate MLP + on-device softmax -> gw ----
            with tc.tile_pool(name="gate_sb", bufs=1) as gsb, \
                 tc.tile_pool(name="gate_psum", bufs=1, space="PSUM") as gps:
                wd_sb = gsb.tile([128, HB, D], BF16)
                nc.sync.dma_start(wd_sb[:], wd_ap)
                wu_sb = gsb.tile([128, D // 128, K], BF16)
                nc.sync.dma_start(wu_sb[:], wu_ap)
                bd_sb = gsb.tile([128, D // 128, 1], F32)
                nc.sync.dma_start(bd_sb[:], bd_ap)
                bu_sb = gsb.tile([K, 1], F32)
                nc.sync.dma_start(bu_sb[:], bu_d[:])
                gT = gsb.tile([128, D // 128, T], BF16)
                for d in range(D // 128):
                    pg = gps.tile([128, T], F32, tag="pg", name=f"pg{d}", bufs=2)
                    for h in range(HB):
                        for half in range(2):
                            nc.tensor.matmul(
                                pg[:, half * 512 : (half + 1) * 512],
                                lhsT=wd_sb[:, h, d * 128 : (d + 1) * 128],
                                rhs=xT[:, h, half * 512 : (half + 1) * 512],
                                start=(h == 0), stop=(h == HB - 1),
                            )
                    lin = gsb.tile([128, T], F32, tag="lin", name=f"lin{d}")
                    nc.scalar.activation(lin[:], pg[:], AFT.Identity,
                                         bias=bd_sb[:, d, :], scale=1.0)
                    sig = gsb.tile([128, T], F32, tag="sig", name=f"sig{d}")
                    nc.scalar.activation(sig[:], pg[:], AFT.Sigmoid,
                                         bias=bd_sb[:, d, :], scale=1.0)
                    nc.vector.tensor_mul(gT[:, d, :], lin[:], sig[:])
                pl = gps.tile([K, T], F32, tag="pl", bufs=1)
                for d in range(D // 128):
                    for half in range(2):
                        nc.tensor.matmul(
                            pl[:, half * 512 : (half + 1) * 512],
                            lhsT=wu_sb[:, d, :],
                            rhs=gT[:, d, half * 512 : (half + 1) * 512],
                            start=(d == 0), stop=(d == D // 128 - 1),
                        )
                gl_sb = gsb.tile([K, T], F32)
                nc.scalar.activation(gl_sb[:], pl[:], AFT.Identity,
                                     bias=bu_sb[:], scale=1.0)
                # softmax over K: transpose to t-major then rowwise ops
                glt = gsb.tile([128, TB, K], F32)
                for t in range(TB):
                    gp = gps.tile([128, K], F32, tag="gp", name=f"gp{t}", bufs=2)
                    nc.tensor.transpose(gp[:], gl_sb[:, t * 128 : (t + 1) * 128],
                                        ident32[:4, :4])
                    nc.vector.tensor_copy(glt[:, t, :], gp[:])
                negm = gsb.tile([128, TB], F32)
                esum = gsb.tile([128, TB], F32)
                for t in range(TB):
                    nc.vector.tensor_reduce(
                        negm[:, t : t + 1], glt[:, t, :],
                        axis=mybir.AxisListType.X, op=mybir.AluOpType.max,
                        negate=True,
                    )
                    nc.scalar.activation(gw[:, t, :], glt[:, t, :], AFT.Exp,
                                         bias=negm[:, t : t + 1], scale=1.0,
                                         accum_out=esum[:, t : t + 1])
                rsum = gsb.tile([128, TB], F32)
                nc.vector.reciprocal(rsum[:], esum[:])
                for t in range(TB):
                    nc.vector.tensor_scalar_mul(gw[:, t, :], gw[:, t, :],
                                                rsum[:, t : t + 1])

            # ---- fused GEMM + exp + CC + mix; smaller passes at the end so
            # the final AllReduce + mix expose as little tail as possible ----
            PASSES = [(0, 2), (2, 2), (4, 2), (6, 2)]
            with tc.tile_pool(name="wmm", bufs=3) as wpool, \
                 tc.tile_pool(name="pfull", bufs=4) as ppool, \
                 tc.tile_pool(name="mix", bufs=2) as mixp, \
                 tc.tile_pool(name="ccdr", bufs=len(PASSES), space="DRAM") as ccdr, \
                 tc.tile_pool(name="mm_psum", bufs=2, space="PSUM") as mmps:
                for q, (ts, cnt) in enumerate(PASSES):
                    pts = []
                    for t2 in range(cnt):
                        pts.append(ppool.tile([128, C], BF16, tag="P",
                                              name=f"P{q}_{t2}"))
                    schunk = mixp.tile([128, cnt, NG], F32, tag="schunk",
                                       name=f"sch{q}")
                    NCH = 4 // cnt          # psum chunks per t-block per step
                    GW_COLS = 512 * NCH
                    for gg in range(C // (512 * 4 // cnt) // cnt):
                        pass  # placeholder
                    for gg in range(NG // NCH):
                        psums = []
                        for t2 in range(cnt):
                            for ch in range(NCH):
                                psums.append(mmps.tile([128, 512], F32,
                                                       tag=f"mm{t2 * NCH + ch}",
                                                       name=f"mm_{q}_{gg}_{t2}_{ch}"))
                        for hs in range(HB // 2):
                            wt = wpool.tile([128, 2, GW_COLS], FP8, tag="wt",
                                            name=f"wt{q}_{gg}_{hs}")
                            nc.sync.dma_start(
                                wt[:], w_ap8[hs, :, :, gg * GW_COLS : (gg + 1) * GW_COLS])
                            for t2 in range(cnt):
                                t = ts + t2
                                for ch in range(NCH):
                                    nc.tensor.matmul(
                                        psums[t2 * NCH + ch][:],
                                        lhsT=xT8[:, 2 * hs : 2 * hs + 2, t * 128 : (t + 1) * 128],
                                        rhs=wt[:, :, ch * 512 : (ch + 1) * 512],
                                        start=(hs == 0), stop=(hs == HB // 2 - 1),
                                        perf_mode=mybir.MatmulPerfMode.DoubleRow,
                                    )
                        for t2 in range(cnt):
                            for ch in range(NCH):
                                g = gg * NCH + ch
                                nc.scalar.activation(pts[t2][:, g * 512 : (g + 1) * 512],
                                                     psums[t2 * NCH + ch][:], AFT.Exp,
                                                     bias=0.0, scale=1.0 / WSCALE,
                                                     accum_out=schunk[:, t2, g : g + 1])
                    # local sums -> AllReduce -> a = gw / Z
                    s_q = mixp.tile([128, cnt, K], F32, tag="s_q", name=f"s_q{q}")
                    nc.vector.tensor_reduce(
                        s_q[:], schunk[:].rearrange("p t (k g) -> p t k g", g=GPE),
                        axis=mybir.AxisListType.X, op=mybir.AluOpType.add,
                    )
                    bi = ccdr.tile([128, cnt * K], F32, tag=f"bi{cnt}", name=f"bi{q}")
                    bo = ccdr.tile([128, cnt * K], F32, tag=f"bo{cnt}", name=f"bo{q}")
                    nc.sync.dma_start(bi[:],
                                      s_q[:].rearrange("p t k -> p (t k)"))
                    nc.gpsimd.collective_compute(
                        "AllReduce", mybir.AluOpType.add,
                        replica_groups=[list(range(NC))],
                        ins=[bi[:]], outs=[bo[:]],
                    )
                    z_q = mixp.tile([128, cnt, K], F32, tag="z_q", name=f"z_q{q}")
                    nc.sync.dma_start(z_q[:].rearrange("p t k -> p (t k)"),
                                      bo[:])
                    nc.vector.tensor_scalar_add(z_q[:], z_q[:],
                                                -float((VP - VSH) * NC))
                    a_q = mixp.tile([128, cnt, K], F32, tag="a_q", name=f"a_q{q}")
                    nc.vector.reciprocal(a_q[:], z_q[:])
                    nc.vector.tensor_mul(a_q[:], a_q[:],
                                         gw[:, ts : ts + cnt, :])
                    # mix + log + out
                    for t2 in range(cnt):
                        t = ts + t2
                        red = mixp.tile([128, VP], BF16, tag="red", name=f"red{t}")
                        mks = []
                        for k in range(K):
                            pk = pts[t2][:, k * VP : (k + 1) * VP]
                            if k == 0:
                                nc.vector.tensor_scalar_mul(red[:], pk, a_q[:, t2, 0:1])
                            else:
                                mk = mixp.tile([128, VP], BF16, tag="mk",
                                               name=f"mk{t}_{k}", bufs=1)
                                nc.vector.tensor_scalar_mul(mk[:], pk, a_q[:, t2, k : k + 1])
                                mks.append(mk)
                        for mk in mks:
                            nc.vector.tensor_add(red[:], red[:], mk[:])
                        ot = mixp.tile([128, VSH], F32, tag="ot", name=f"ot{t}",
                                       bufs=1)
                        nc.scalar.activation(ot[:], red[:, :VSH], AFT.Ln,
                                             bias=eps_log[:], scale=1.0)
                        nc.sync.dma_start(o_d[t], ot[:])
    nc.compile()
    return nc


def build_phase_b():
    nc = bacc.Bacc("TRN2", target_bir_lowering=False, debug=False, num_devices=NC)
    p_d = nc.dram_tensor("p", [TB, 128, C], BF16, kind="ExternalInput")
    a_d = nc.dram_tensor("a", [TB, 128, K], F32, kind="ExternalInput")
    o_d = nc.dram_tensor("o", [TB, 128, VSH], F32, kind="ExternalOutput")

    with tile.TileContext(nc) as tc:
        with tc.tile_pool(name="sb", bufs=2) as sb, \
             tc.tile_pool(name="pkp", bufs=6) as pkp, \
             tc.tile_pool(name="red", bufs=3) as redp:
            eps_log = sb.tile([128, 1], F32, tag="epsl", bufs=1)
            nc.gpsimd.memset(eps_log[:], EPS_LOG)
            for t in range(TB):
                at = sb.tile([128, K], F32, tag="at")
                nc.sync.dma_start(at[:], a_d[t])
                red = redp.tile([128, VP], BF16, tag="red")
                mks = []
                for k in range(K):
                    pk = pkp.tile([128, VP], BF16, tag="pk", name=f"pk{t}_{k}")
                    nc.sync.dma_start(pk[:], p_d[t, :, k * VP : (k + 1) * VP])
                    if k == 0:
                        nc.vector.tensor_scalar_mul(red[:], pk[:], at[:, 0:1])
                    else:
                        mk = sb.tile([128, VP], BF16, tag="mk", name=f"mk{t}_{k}")
                        nc.vector.tensor_scalar_mul(mk[:], pk[:], at[:, k : k + 1])
                        mks.append(mk)
                for mk in mks:
                    nc.vector.tensor_tensor(red[:], red[:], mk[:],
                                            op=mybir.AluOpType.add)
                ot = redp.tile([128, VSH], F32, tag="ot")
                nc.scalar.activation(ot[:], red[:, :VSH], AFT.Ln,
                                     bias=eps_log[:], scale=1.0)
                nc.sync.dma_start(o_d[t], ot[:])
    nc.compile()
    return nc


_CACHE = {}


def _get_kernels():
    if "f" not in _CACHE:
        _CACHE["f"] = build_fused()
    return _CACHE["f"]


def kernel(hidden_states, rms_scale, gate_down_w, gate_down_b, gate_up_w,
           gate_up_b, expert_w, trace=False):
    nc_f = _get_kernels()
    core_ids = list(range(NC))

    x = np.ascontiguousarray(np.asarray(hidden_states, dtype=np.float32).reshape(T, H))
    scale = np.asarray(rms_scale, dtype=np.float32)
    # fold rms_scale into every weight that consumes the normed activations
    wd = (np.asarray(gate_down_w, dtype=np.float32) * scale[:, None]).astype(ml_dtypes.bfloat16)
    wu = np.asarray(gate_up_w, dtype=np.float32).astype(ml_dtypes.bfloat16)
    bd = np.ascontiguousarray(np.asarray(gate_down_b, dtype=np.float32).reshape(D, 1))
    bu = np.ascontiguousarray(np.asarray(gate_up_b, dtype=np.float32).reshape(K, 1))
    we = np.asarray(expert_w, dtype=np.float32) * scale[:, None]

    in_maps = []
    for c in range(NC):
        wsh = np.zeros((H, C), dtype=FP8NP)
        for k in range(K):
            wsh[:, k * VP : k * VP + VSH] = (
                we[:, k * V + c * VSH : k * V + (c + 1) * VSH] * WSCALE
            ).astype(FP8NP)
        in_maps.append({"x": x, "w": wsh, "wd": wd, "wu": wu, "bd": bd, "bu": bu})

    res = run_bass_kernel_spmd(nc_f, in_maps, core_ids, trace=trace)

    out = np.empty((T, V), dtype=np.float32)
    for c in range(NC):
        out[:, c * VSH : (c + 1) * VSH] = res.results[c]["o"].reshape(T, VSH)
    out = out.reshape(B, S, V)
    if trace:
        return out, (res, res)
    return out



# revision 2
# speedup vs baseline: 1.5305x; 1.5305x over previous
"""Trainium2 Bass kernel for MixtureOfSoftmaxes (v2).

Module: RMSNorm -> gate MLP (silu, softmax over K experts) -> big GEMM
x @ expert_w (H=1024 -> K*V=128000), softmax over V per expert, mix with
gate weights, log.

Sharding: tensor-parallel over vocab. Core c computes, for all K=4
experts, the vocab window [c*4000, (c+1)*4000) (no padding). The only
cross-core quantity is the per-(token, expert) softmax denominator
Z = sum_v exp(logit), AllReduced per sweep (4 sweeps x 2 token blocks).

Single fused program per core:
  preamble: load x, RMSNorm folded into the transpose (diag(rinv) rhs),
    xT8 fp8 [128, HB, T]; gate MLP in fp8 off xT8 -> gw (softmax over K).
  4 sweeps over token-block pairs: stream W fp8 (fat 4KB descriptors),
    DoubleRow matmuls into [128,1024] psum tiles (8 banks ping-pong),
    1024-wide exp epilogue -> P bf16 resident in SBUF + row-sum accums.
    Per sweep: AllReduce of row sums (overlapped with next sweep), then
    a = gw/Z, in-place DVE mix (scalar_tensor_tensor), Ln, bf16 out.
"""

import sys

sys.path.insert(0, "/opt/trn_rl_repo")

import numpy as np
import ml_dtypes

import concourse.bass as bass
import concourse.bacc as bacc
import concourse.mybir as mybir
import concourse.tile as tile
from concourse.bass_utils import run_bass_kernel_spmd
from concourse.masks import make_identity

AFT = mybir.ActivationFunctionType
ALU = mybir.AluOpType
F32 = mybir.dt.float32
BF16 = mybir.dt.bfloat16
FP8 = mybir.dt.float8e4
FP8NP = ml_dtypes.float8_e4m3
WSCALE = 16.0

B, S, H, K, V = 2, 512, 1024, 4, 32000
T = B * S              # 1024 tokens
NC = 8                 # cores
VSH = V // NC          # 4000 vocab cols per core per expert
C = K * VSH            # 16000 GEMM cols per core
D = H // 2             # 512 gate hidden
EPS_RMS = 1e-5
EPS_LOG = 1e-10
TB = T // 128          # 8 token blocks
HB = H // 128          # 8 contraction blocks
HS = HB // 2           # 4 double-row contraction steps
WA = 2048              # first super-chunk per expert
WB = VSH - WA          # 1952 second super-chunk
NSW = 4                # sweeps
NBS = TB // NSW        # 2 token blocks per sweep
ACC_PE = 4             # accum units per expert (1024,1024,1024,928)


def build():
    nc = bacc.Bacc("TRN2", target_bir_lowering=False, debug=False, num_devices=NC)
    x_d = nc.dram_tensor("x", [T, H], F32, kind="ExternalInput")
    wa_d = nc.dram_tensor("wa", [K, HS, 128, 2, WA], FP8, kind="ExternalInput")
    wb_d = nc.dram_tensor("wb", [K, HS, 128, 2, WB], FP8, kind="ExternalInput")
    wd_d = nc.dram_tensor("wd", [HS, 128, 2, D], FP8, kind="ExternalInput")
    wu_d = nc.dram_tensor("wu", [D, K], BF16, kind="ExternalInput")
    bd_d = nc.dram_tensor("bd", [D, 1], F32, kind="ExternalInput")
    bu_d = nc.dram_tensor("bu", [K, 1], F32, kind="ExternalInput")
    o_d = nc.dram_tensor("o", [TB, 128, VSH], BF16, kind="ExternalOutput")

    x_ap = x_d.rearrange("(t p) h -> t p h", p=128)
    wd_ap = wd_d.rearrange("hs p j d -> p hs j d")
    wu_ap = wu_d.rearrange("(db p) k -> p db k", p=128)
    bd_ap = bd_d.rearrange("(db p) o -> p db o", p=128)

    with tile.TileContext(nc) as tc:
        with tc.tile_pool(name="persist", bufs=1) as pers:
            ident = pers.tile([128, 128], BF16)
            make_identity(nc, ident[:])
            ident32 = pers.tile([128, 128], F32)
            make_identity(nc, ident32[:])
            eps_rms = pers.tile([128, 1], F32)
            nc.gpsimd.memset(eps_rms[:], EPS_RMS)
            eps_log = pers.tile([128, 1], F32)
            nc.gpsimd.memset(eps_log[:], EPS_LOG)
            xT8 = pers.tile([128, HB, T], FP8)   # 8 KB/partition
            ss = pers.tile([128, TB], F32)
            sd = pers.tile([128, TB], F32)
            rinv = pers.tile([128, TB], F32)
            gw = pers.tile([128, TB, K], F32)

            # ---- preamble: load + RMSNorm + normalizing transpose ----
            with tc.tile_pool(name="norm", bufs=2) as norm_pool, \
                 tc.tile_pool(name="tp_psum", bufs=4, space="PSUM") as tp_psum:
                for t in range(TB):
                    xt = norm_pool.tile([128, H], F32, tag="xt")
                    nc.sync.dma_start(xt[:], x_ap[t])
                    sq = norm_pool.tile([128, H], F32, tag="sq")
                    nc.scalar.activation(sq[:], xt[:], AFT.Square, bias=0.0,
                                         scale=1.0, accum_out=ss[:, t : t + 1])
                    nc.scalar.activation(sd[:, t : t + 1], ss[:, t : t + 1],
                                         AFT.Sqrt, bias=eps_rms[:], scale=1.0 / H)
                    nc.vector.reciprocal(rinv[:, t : t + 1], sd[:, t : t + 1])
                    xb = norm_pool.tile([128, H], BF16, tag="xb")
                    nc.vector.tensor_copy(xb[:], xt[:])
                    diag = norm_pool.tile([128, 128], BF16, tag="diag")
                    nc.vector.tensor_scalar_mul(diag[:], ident[:],
                                                rinv[:, t : t + 1])
                    # tp[h, t'] = sum_t xb[t, h] * diag[t, t'] = xnorm^T
                    for h in range(HB):
                        tp = tp_psum.tile([128, 128], F32, tag="tp")
                        nc.tensor.matmul(tp[:], lhsT=xb[:, h * 128 : (h + 1) * 128],
                                         rhs=diag[:], start=True, stop=True)
                        if h % 2 == 0:
                            nc.scalar.copy(xT8[:, h, t * 128 : (t + 1) * 128], tp[:])
                        else:
                            nc.vector.tensor_copy(xT8[:, h, t * 128 : (t + 1) * 128], tp[:])

            # ---- gate MLP (fp8, DoubleRow) + softmax over K -> gw ----
            with tc.tile_pool(name="gate_sb", bufs=1) as gsb, \
                 tc.tile_pool(name="gate_psum", bufs=1, space="PSUM") as gps:
                wd_sb = gsb.tile([128, HS, 2, D], FP8)
                nc.sync.dma_start(wd_sb[:], wd_ap)
                wu_sb = gsb.tile([128, D // 128, K], BF16)
                nc.sync.dma_start(wu_sb[:], wu_ap)
                bd_sb = gsb.tile([128, D // 128, 1], F32)
                nc.sync.dma_start(bd_sb[:], bd_ap)
                bu_sb = gsb.tile([K, 1], F32)
                nc.sync.dma_start(bu_sb[:], bu_d[:])
                gT = gsb.tile([128, D // 128, T], BF16)
                for d in range(D // 128):
                    pg = gps.tile([128, T], F32, tag="pg", name=f"pg{d}", bufs=2)
                    for hs in range(HS):
                        for half in range(2):
                            nc.tensor.matmul(
                                pg[:, half * 512 : (half + 1) * 512],
                                lhsT=wd_sb[:, hs, :, d * 128 : (d + 1) * 128],
                                rhs=xT8[:, 2 * hs : 2 * hs + 2,
                                        half * 512 : (half + 1) * 512],
                                start=(hs == 0), stop=(hs == HS - 1),
                                perf_mode=mybir.MatmulPerfMode.DoubleRow,
                            )
                    lin = gsb.tile([128, T], F32, tag="lin", name=f"lin{d}")
                    nc.scalar.activation(lin[:], pg[:], AFT.Identity,
                                         bias=bd_sb[:, d, :], scale=1.0 / WSCALE)
                    sig = gsb.tile([128, T], F32, tag="sig", name=f"sig{d}")
                    nc.scalar.activation(sig[:], pg[:], AFT.Sigmoid,
                                         bias=bd_sb[:, d, :], scale=1.0 / WSCALE)
                    nc.vector.tensor_mul(gT[:, d, :], lin[:], sig[:])
                pl = gps.tile([K, T], F32, tag="pl", bufs=1)
                for d in range(D // 128):
                    for half in range(2):
                        nc.tensor.matmul(
                            pl[:, half * 512 : (half + 1) * 512],
                            lhsT=wu_sb[:, d, :],
                            rhs=gT[:, d, half * 512 : (half + 1) * 512],
                            start=(d == 0), stop=(d == D // 128 - 1),
                        )
                gl_sb = gsb.tile([K, T], F32)
                nc.scalar.activation(gl_sb[:], pl[:], AFT.Identity,
                                     bias=bu_sb[:], scale=1.0)
                # softmax over K: transpose to t-major then rowwise ops
                glt = gsb.tile([128, TB, K], F32)
                for t in range(TB):
                    gp = gps.tile([128, K], F32, tag="gp", name=f"gp{t}", bufs=2)
                    nc.tensor.transpose(gp[:], gl_sb[:, t * 128 : (t + 1) * 128],
                                        ident32[:4, :4])
                    nc.vector.tensor_copy(glt[:, t, :], gp[:])
                negm = gsb.tile([128, TB], F32)
                esum = gsb.tile([128, TB], F32)
                for t in range(TB):
                    nc.vector.tensor_reduce(
                        negm[:, t : t + 1], glt[:, t, :],
                        axis=mybir.AxisListType.X, op=ALU.max, negate=True,
                    )
                    nc.scalar.activation(gw[:, t, :], glt[:, t, :], AFT.Exp,
                                         bias=negm[:, t : t + 1], scale=1.0,
                                         accum_out=esum[:, t : t + 1])
                rsum = gsb.tile([128, TB], F32)
                nc.vector.reciprocal(rsum[:], esum[:])
                for t in range(TB):
                    nc.vector.tensor_scalar_mul(gw[:, t, :], gw[:, t, :],
                                                rsum[:, t : t + 1])

            # ---- sweeps: GEMM + exp + AllReduce + mix (software pipelined) --
            # super-chunk layout per expert: [0:2048] from wa, [2048:4000] wb
            supers = [(0, WA), (WA, WB)]   # (col offset in expert, width)

            def emit_sweep(s, P_tiles, mixp, wpool, mmps, ccdr):
                """GEMM + exp for token blocks 2s, 2s+1; returns (schunk, bi, bo)."""
                schunk = mixp.tile([128, NBS, K * ACC_PE], F32, tag="schunk",
                                   name=f"sch{s}")
                for k in range(K):
                    for sup, (coff, wid) in enumerate(supers):
                        w_src = (wa_d if sup == 0 else wb_d)
                        wts = []
                        for hs in range(HS):
                            wt = wpool.tile([128, 2, WA], FP8, tag=f"wt{hs}",
                                            name=f"wt{s}_{k}_{sup}_{hs}")
                            nc.sync.dma_start(wt[:, :, :wid], w_src[k, hs])
                            wts.append(wt)
                        for half in range(2):
                            cw = 1024 if half == 0 else wid - 1024  # 1024 or 928
                            pss = []
                            for t2 in range(NBS):
                                ps = mmps.tile([128, 1024], F32, tag=f"ps{t2}",
                                               name=f"ps{s}_{k}_{sup}_{half}_{t2}")
                                pss.append(ps)
                            for hs in range(HS):
                                for t2 in range(NBS):
                                    t = s * NBS + t2
                                    for sl in range(0, cw, 512):
                                        sw = min(512, cw - sl)
                                        nc.tensor.matmul(
                                            pss[t2][:, sl : sl + sw],
                                            lhsT=xT8[:, 2 * hs : 2 * hs + 2,
                                                     t * 128 : (t + 1) * 128],
                                            rhs=wts[hs][:, :,
                                                        half * 1024 + sl
                                                        : half * 1024 + sl + sw],
                                            start=(hs == 0), stop=(hs == HS - 1),
                                            perf_mode=mybir.MatmulPerfMode.DoubleRow,
                                        )
                            acc_i = k * ACC_PE + sup * 2 + half
                            for t2 in range(NBS):
                                col = k * VSH + coff + half * 1024
                                nc.scalar.activation(
                                    P_tiles[t2][:, col : col + cw],
                                    pss[t2][:, :cw], AFT.Exp,
                                    bias=0.0, scale=1.0 / WSCALE,
                                    accum_out=schunk[:, t2, acc_i : acc_i + 1])
                # local row sums -> DRAM for AllReduce
                s_sb = mixp.tile([128, NBS, K], F32, tag="s_sb", name=f"s_sb{s}")
                nc.vector.tensor_reduce(
                    s_sb[:], schunk[:].rearrange("p t (k g) -> p t k g", g=ACC_PE),
                    axis=mybir.AxisListType.X, op=ALU.add,
                )
                bi = ccdr.tile([128, NBS * K], F32, tag="bi", name=f"bi{s}")
                bo = ccdr.tile([128, NBS * K], F32, tag="bo", name=f"bo{s}")
                nc.sync.dma_start(bi[:], s_sb[:].rearrange("p t k -> p (t k)"))
                nc.gpsimd.collective_compute(
                    "AllReduce", ALU.add,
                    replica_groups=[list(range(NC))],
                    ins=[bi[:]], outs=[bo[:]],
                )
                return bo

            def emit_mix(s, P_tiles, mixp, bo):
                """z -> a -> in-place mix -> Ln -> out DMA for sweep s."""
                z_q = mixp.tile([128, NBS, K], F32, tag="z_q", name=f"z_q{s}")
                nc.sync.dma_start(z_q[:].rearrange("p t k -> p (t k)"), bo[:])
                a_q = mixp.tile([128, NBS, K], F32, tag="a_q", name=f"a_q{s}")
                nc.vector.reciprocal(a_q[:], z_q[:])
                nc.vector.tensor_mul(a_q[:], a_q[:],
                                     gw[:, s * NBS : (s + 1) * NBS, :])
                for t2 in range(NBS):
                    t = s * NBS + t2
                    Pt = P_tiles[t2]
                    nc.vector.tensor_scalar_mul(Pt[:, 0:VSH], Pt[:, 0:VSH],
                                                a_q[:, t2, 0:1])
                    for k in range(1, K):
                        nc.vector.scalar_tensor_tensor(
                            Pt[:, 0:VSH], Pt[:, k * VSH : (k + 1) * VSH],
                            a_q[:, t2, k : k + 1], Pt[:, 0:VSH],
                            op0=ALU.mult, op1=ALU.add,
                        )
                    ot = mixp.tile([128, VSH], BF16, tag="ot", name=f"ot{t}")
                    nc.scalar.activation(ot[:], Pt[:, 0:VSH], AFT.Ln,
                                         bias=eps_log[:], scale=1.0)
                    nc.sync.dma_start(o_d[t], ot[:])

            with tc.tile_pool(name="pP", bufs=4) as pP, \
                 tc.tile_pool(name="wmm", bufs=2) as wpool, \
                 tc.tile_pool(name="mix", bufs=2) as mixp, \
                 tc.tile_pool(name="ccdr", bufs=4, space="DRAM") as ccdr, \
                 tc.tile_pool(name="mm_psum", bufs=2, space="PSUM") as mmps:
                bos = {}
                P_all = {}
                for s in range(NSW):
                    P_tiles = [pP.tile([128, C], BF16, tag="P",
                                       name=f"P{s}_{t2}") for t2 in range(NBS)]
                    P_all[s] = P_tiles
                    bos[s] = emit_sweep(s, P_tiles, mixp, wpool, mmps, ccdr)
                    if s >= 1:
                        emit_mix(s - 1, P_all[s - 1], mixp, bos[s - 1])
                emit_mix(NSW - 1, P_all[NSW - 1], mixp, bos[NSW - 1])
    nc.compile()
    return nc


_CACHE = {}


def _get_kernel():
    if "k" not in _CACHE:
        _CACHE["k"] = build()
    return _CACHE["k"]


def kernel(hidden_states, rms_scale, gate_down_w, gate_down_b, gate_up_w,
           gate_up_b, expert_w, trace=False):
    nc_k = _get_kernel()
    core_ids = list(range(NC))

    x = np.ascontiguousarray(
        np.asarray(hidden_states, dtype=np.float32).reshape(T, H))
    scale = np.asarray(rms_scale, dtype=np.float32)
    # fold rms_scale into every weight that consumes the normed activations
    wd_f = np.asarray(gate_down_w, dtype=np.float32) * scale[:, None]
    # gate weights in fp8 (scaled by WSCALE, undone in the silu epilogue)
    wd8 = np.ascontiguousarray(
        (wd_f * WSCALE).reshape(HS, 2, 128, D).transpose(0, 2, 1, 3)
    ).astype(FP8NP)
    wu = np.asarray(gate_up_w, dtype=np.float32).astype(ml_dtypes.bfloat16)
    bd = np.ascontiguousarray(
        np.asarray(gate_down_b, dtype=np.float32).reshape(D, 1))
    bu = np.ascontiguousarray(
        np.asarray(gate_up_b, dtype=np.float32).reshape(K, 1))
    we = np.asarray(expert_w, dtype=np.float32) * (scale[:, None] * WSCALE)

    in_maps = []
    for c in range(NC):
        wa = np.empty((K, HS, 128, 2, WA), dtype=FP8NP)
        wb = np.empty((K, HS, 128, 2, WB), dtype=FP8NP)
        for k in range(K):
            blk = we[:, k * V + c * VSH : k * V + (c + 1) * VSH]
            fr = blk.reshape(HS, 2, 128, VSH).transpose(0, 2, 1, 3)
            wa[k] = fr[:, :, :, :WA].astype(FP8NP)
            wb[k] = fr[:, :, :, WA:].astype(FP8NP)
        in_maps.append({"x": x, "wa": wa, "wb": wb, "wd": wd8, "wu": wu,
                        "bd": bd, "bu": bu})

    res = run_bass_kernel_spmd(nc_k, in_maps, core_ids, trace=trace)

    out = np.empty((T, V), dtype=np.float32)
    for c in range(NC):
        out[:, c * VSH : (c + 1) * VSH] = (
            res.results[c]["o"].astype(np.float32).reshape(T, VSH))
    out = out.reshape(B, S, V)
    if trace:
        return out, (res, res)
    return out
